# revision 3
# baseline (speedup 1.0000x reference)
"""Trainium2 Bass kernel for nn_LoRALinear (quantized linear + LoRA).

reference:
    w_dq = quant_dequant_int8_per_row(weight)          # [out, in]
    out  = x @ w_dq.T + (alpha/r) * (x @ la) @ lb      # [T, out]

Math identity used here:
    out = x @ (w_dq.T + 2.0 * (la @ lb)) = x @ W_eff

The quant-dequant + LoRA fold is cheap elementwise/skinny-matmul host prep;
the 550-GFLOP dense matmul runs on 8 NeuronCores, data-parallel over tokens.

Device kernel (per core, SPMD identical program):
    xt  [128, 8, 32, 256]  bf16  - token shard, [p][mc][ko][mi] layout
    w   [8, 128, 32, 512]  bf16  - W_eff, [n0][p][ko][ns] layout, replicated
    out [8, 2, 128, 8, 512] bf16 - [n0][g][p][m8][ns] layout

All DRAM<->SBUF transfers are per-partition contiguous (8-32 KiB per
descriptor) so DMA runs at line rate instead of being descriptor-bound.
Matmuls accumulate in fp32 PSUM; the bf16 store adds ~1e-3 relative
rounding on top of the ~2e-3 from bf16 operands (gate is 2e-2).
"""

import numpy as np
import ml_dtypes

TOKENS, IN_F, OUT_F, R = 16384, 4096, 4096, 16
N_CORES = 8
TPC = TOKENS // N_CORES  # tokens per core: 2048
SCALING = 2.0  # alpha / r
P = 128
NS = 512   # out_feature stripe (one PSUM bank of f32)
MC = 256   # x fill chunk: tokens per chunk
MG = 8     # m-tiles per output store group

_NC_CACHE = {}


def _build_nc(tpc=TPC, in_f=IN_F, out_f=OUT_F, ns=NS):
    import concourse.mybir as mybir
    import concourse.tile as tile
    from concourse import bacc

    nc = bacc.Bacc("TRN2", target_bir_lowering=False)

    ko_n = in_f // P    # k-outer tiles (32)
    mt_n = tpc // P     # token tiles (16)
    nt_n = out_f // ns  # out_f stripes (8)
    mc_n = tpc // MC    # x fill chunks (8)
    mg_n = mt_n // MG   # output store groups per stripe (2)

    xt = nc.dram_tensor(
        "xt", [P, mc_n, ko_n, MC], mybir.dt.bfloat16, kind="ExternalInput"
    )
    w = nc.dram_tensor(
        "w", [nt_n, P, ko_n, ns], mybir.dt.bfloat16, kind="ExternalInput"
    )
    out = nc.dram_tensor(
        "out", [nt_n, mg_n, P, MG, ns], mybir.dt.bfloat16, kind="ExternalOutput"
    )

    with tile.TileContext(nc) as tc:
        with (
            tc.tile_pool(name="xpool", bufs=1) as xpool,
            tc.tile_pool(name="wpool", bufs=2) as wpool,
            tc.tile_pool(name="opool", bufs=1) as opool,
            tc.tile_pool(name="pspool", bufs=4, space="PSUM") as pspool,
        ):
            # Whole x shard stays resident in SBUF (bf16: 128 KiB/partition).
            x_sb = xpool.tile([P, mc_n, ko_n, MC], mybir.dt.bfloat16)

            # Stripe 0 of W and the x chunks are interleaved so the first
            # psum group's matmuls wait only on the first chunks, not the
            # whole 21 MB: PE starts a few us in, DMA streams under compute.
            kc_n = 4  # stripe-0 ko chunks
            kcs = ko_n // kc_n
            w_sb0 = wpool.tile([P, ko_n, ns], mybir.dt.bfloat16, name="w_sb")
            issue = (
                [("w0", 0), ("x", 0), ("w0", 1), ("x", 1), ("w0", 2), ("w0", 3)]
                + [("x", i) for i in range(2, mc_n)]
            )
            for kind, i in issue:
                if kind == "w0":
                    nc.sync.dma_start(
                        w_sb0[:, i * kcs : (i + 1) * kcs, :],
                        w[0, :, i * kcs : (i + 1) * kcs, :],
                    )
                else:
                    nc.sync.dma_start(x_sb[:, i], xt[:, i])

            for n in range(nt_n):
                if n == 0:
                    w_sb = w_sb0
                else:
                    w_sb = wpool.tile([P, ko_n, ns], mybir.dt.bfloat16, name="w_sb")
                    nc.sync.dma_start(w_sb[:], w[n])
                for g in range(mg_n):
                    o_sb = opool.tile([P, MG, ns], mybir.dt.bfloat16)
                    for m8 in range(MG):
                        m = g * MG + m8
                        mc, mi = divmod(m * P, MC)
                        ps = pspool.tile([P, ns], mybir.dt.float32)
                        for ko in range(ko_n):
                            nc.tensor.matmul(
                                ps[:],
                                x_sb[:, mc, ko, mi : mi + P],
                                w_sb[:, ko, :],
                                start=(ko == 0),
                                stop=(ko == ko_n - 1),
                            )
                        nc.vector.tensor_copy(o_sb[:, m8, :], ps[:])
                    nc.sync.dma_start(out[n, g], o_sb[:])

    nc.finalize()
    return nc


def _host_prep(x, weight, lora_a, lora_b):
    x = np.asarray(x, dtype=np.float32)
    weight = np.asarray(weight, dtype=np.float32)
    la = np.asarray(lora_a, dtype=np.float32)
    lb = np.asarray(lora_b, dtype=np.float32)

    # Symmetric per-row absmax int8 quant-dequant, matching the reference's
    # fp32 elementwise ops bit-for-bit (max/div/round/clip are exact or
    # correctly rounded in IEEE f32 on any backend).
    abs_max = np.max(np.abs(weight), axis=-1, keepdims=True)
    scale = (abs_max / np.float32(127.0)).astype(np.float32)
    wq = np.clip(
        np.round(weight / (scale + np.float32(1e-8))), -128.0, 127.0
    ).astype(np.float32)
    w_dq = wq * scale

    w_eff = w_dq.T + np.float32(SCALING) * (la @ lb)
    w_bf = w_eff.astype(ml_dtypes.bfloat16)
    # [in, out] -> [n0][p][ko][ns]
    w_dram = np.ascontiguousarray(
        w_bf.reshape(IN_F // P, P, OUT_F // NS, NS).transpose(2, 1, 0, 3)
    )

    x_bf = x.astype(ml_dtypes.bfloat16)
    xt_shards = []
    for c in range(N_CORES):
        xs = x_bf[c * TPC : (c + 1) * TPC]  # [tpc, in_f]
        # [tpc, in_f] -> [p][mc][ko][mi]
        xt_shards.append(
            np.ascontiguousarray(
                xs.reshape(TPC // MC, MC, IN_F // P, P).transpose(3, 0, 2, 1)
            )
        )
    return xt_shards, w_dram


def _unshard(outs):
    # outs: per-core [n0][g][p][m8][ns] bf16 -> full [TOKENS, OUT_F] f32
    full = np.empty((TOKENS, OUT_F), dtype=np.float32)
    for c, o in enumerate(outs):
        y = o.transpose(1, 3, 2, 0, 4).reshape(TPC, OUT_F)
        full[c * TPC : (c + 1) * TPC] = y.astype(np.float32)
    return full


def kernel(x, weight, lora_a, lora_b):
    from concourse.bass_utils import run_bass_kernel_spmd

    xt_shards, w_dram = _host_prep(x, weight, lora_a, lora_b)

    if "nc" not in _NC_CACHE:
        _NC_CACHE["nc"] = _build_nc()
    nc = _NC_CACHE["nc"]

    in_maps = [{"xt": xt_shards[c], "w": w_dram} for c in range(N_CORES)]
    res = run_bass_kernel_spmd(nc, in_maps, core_ids=list(range(N_CORES)))
    return _unshard([res.results[c]["out"] for c in range(N_CORES)])


# revision 4
# speedup vs baseline: 108.4418x; 108.4418x over previous
"""Trainium2 Bass kernel for nn_LoRALinear (quantized linear + LoRA).

reference:
    w_dq = quant_dequant_int8_per_row(weight)          # [out, in]
    out  = x @ w_dq.T + (alpha/r) * (x @ la) @ lb      # [T, out]

Math identity used here:
    out = x @ (w_dq.T + 2.0 * (la @ lb)) = x @ W_eff

The quant-dequant + LoRA fold is cheap elementwise/skinny-matmul host prep;
the 550-GFLOP dense matmul runs on 8 NeuronCores, data-parallel over tokens.

Device kernel (per core, SPMD identical program):
    xt  [128, 8, 32, 256]  bf16  - token shard, [p][mc][ko][mi] layout
    w   [8, 128, 32, 512]  bf16  - W_eff, [n0][p][ko][ns] layout, replicated
    out [8, 2, 128, 8, 512] bf16 - [n0][g][p][m8][ns] layout

All DRAM<->SBUF transfers are per-partition contiguous (8-32 KiB per
descriptor) so DMA runs at line rate instead of being descriptor-bound.
Matmuls accumulate in fp32 PSUM; the bf16 store adds ~1e-3 relative
rounding on top of the ~2e-3 from bf16 operands (gate is 2e-2).
"""

import numpy as np
import ml_dtypes

TOKENS, IN_F, OUT_F, R = 16384, 4096, 4096, 16
N_CORES = 8
TPC = TOKENS // N_CORES  # tokens per core: 2048
SCALING = 2.0  # alpha / r
P = 128
NS = 512   # out_feature stripe (one PSUM bank of f32)
MC = 256   # x fill chunk: tokens per chunk
MG = 8     # m-tiles per output store group

_NC_CACHE = {}


def _build_nc(tpc=TPC, in_f=IN_F, out_f=OUT_F, ns=NS):
    import concourse.mybir as mybir
    import concourse.tile as tile
    from concourse import bacc

    nc = bacc.Bacc("TRN2", target_bir_lowering=False)

    ko_n = in_f // P    # k-outer tiles (32)
    mt_n = tpc // P     # token tiles (16)
    nt_n = out_f // ns  # out_f stripes (8)
    mc_n = tpc // MC    # x fill chunks (8)
    mg_n = mt_n // MG   # output store groups per stripe (2)

    xt = nc.dram_tensor(
        "xt", [P, mc_n, ko_n, MC], mybir.dt.bfloat16, kind="ExternalInput"
    )
    w = nc.dram_tensor(
        "w", [nt_n, P, ko_n, ns], mybir.dt.bfloat16, kind="ExternalInput"
    )
    out = nc.dram_tensor(
        "out", [nt_n, mg_n, P, MG, ns], mybir.dt.bfloat16, kind="ExternalOutput"
    )

    with tile.TileContext(nc) as tc:
        with (
            tc.tile_pool(name="xpool", bufs=1) as xpool,
            tc.tile_pool(name="wpool", bufs=2) as wpool,
            tc.tile_pool(name="opool", bufs=1) as opool,
            tc.tile_pool(name="pspool", bufs=4, space="PSUM") as pspool,
        ):
            # Whole x shard stays resident in SBUF (bf16: 128 KiB/partition).
            x_sb = xpool.tile([P, mc_n, ko_n, MC], mybir.dt.bfloat16)

            # Stripe 0 of W and the x chunks are interleaved so the first
            # psum group's matmuls wait only on the first chunks, not the
            # whole 21 MB: PE starts a few us in, DMA streams under compute.
            kc_n = 4  # stripe-0 ko chunks
            kcs = ko_n // kc_n
            w_sb0 = wpool.tile([P, ko_n, ns], mybir.dt.bfloat16, name="w_sb")
            issue = (
                [("w0", 0), ("x", 0), ("w0", 1), ("x", 1), ("w0", 2), ("w0", 3)]
                + [("x", i) for i in range(2, mc_n)]
            )
            for kind, i in issue:
                if kind == "w0":
                    nc.sync.dma_start(
                        w_sb0[:, i * kcs : (i + 1) * kcs, :],
                        w[0, :, i * kcs : (i + 1) * kcs, :],
                    )
                else:
                    nc.sync.dma_start(x_sb[:, i], xt[:, i])

            for n in range(nt_n):
                if n == 0:
                    w_sb = w_sb0
                else:
                    w_sb = wpool.tile([P, ko_n, ns], mybir.dt.bfloat16, name="w_sb")
                    nc.sync.dma_start(w_sb[:], w[n])
                for g in range(mg_n):
                    o_sb = opool.tile([P, MG, ns], mybir.dt.bfloat16)
                    for m8 in range(MG):
                        m = g * MG + m8
                        mc, mi = divmod(m * P, MC)
                        ps = pspool.tile([P, ns], mybir.dt.float32)
                        for ko in range(ko_n):
                            nc.tensor.matmul(
                                ps[:],
                                x_sb[:, mc, ko, mi : mi + P],
                                w_sb[:, ko, :],
                                start=(ko == 0),
                                stop=(ko == ko_n - 1),
                            )
                        nc.vector.tensor_copy(o_sb[:, m8, :], ps[:])
                    nc.sync.dma_start(out[n, g], o_sb[:])

    nc.finalize()
    return nc


def _host_prep(x, weight, lora_a, lora_b, ns=NS):
    x = np.asarray(x, dtype=np.float32)
    weight = np.asarray(weight, dtype=np.float32)
    la = np.asarray(lora_a, dtype=np.float32)
    lb = np.asarray(lora_b, dtype=np.float32)

    # Symmetric per-row absmax int8 quant-dequant, matching the reference's
    # fp32 elementwise ops bit-for-bit (max/div/round/clip are exact or
    # correctly rounded in IEEE f32 on any backend).
    abs_max = np.max(np.abs(weight), axis=-1, keepdims=True)
    scale = (abs_max / np.float32(127.0)).astype(np.float32)
    wq = np.clip(
        np.round(weight / (scale + np.float32(1e-8))), -128.0, 127.0
    ).astype(np.float32)
    w_dq = wq * scale

    w_eff = w_dq.T + np.float32(SCALING) * (la @ lb)
    w_bf = w_eff.astype(ml_dtypes.bfloat16)
    # [in, out] -> [n0][p][ko][ns]
    w_dram = np.ascontiguousarray(
        w_bf.reshape(IN_F // P, P, OUT_F // ns, ns).transpose(2, 1, 0, 3)
    )

    x_bf = x.astype(ml_dtypes.bfloat16)
    xt_shards = []
    for c in range(N_CORES):
        xs = x_bf[c * TPC : (c + 1) * TPC]  # [tpc, in_f]
        # [tpc, in_f] -> [p][mc][ko][mi]
        xt_shards.append(
            np.ascontiguousarray(
                xs.reshape(TPC // MC, MC, IN_F // P, P).transpose(3, 0, 2, 1)
            )
        )
    return xt_shards, w_dram


def _unshard(outs):
    # outs: per-core [n0][g][p][m8][ns] bf16 -> full [TOKENS, OUT_F] f32
    full = np.empty((TOKENS, OUT_F), dtype=np.float32)
    for c, o in enumerate(outs):
        y = o.transpose(1, 3, 2, 0, 4).reshape(TPC, OUT_F)
        full[c * TPC : (c + 1) * TPC] = y.astype(np.float32)
    return full


def kernel(x, weight, lora_a, lora_b):
    from concourse.bass_utils import run_bass_kernel_spmd

    xt_shards, w_dram = _host_prep(x, weight, lora_a, lora_b)

    if "nc" not in _NC_CACHE:
        _NC_CACHE["nc"] = _build_nc()
    nc = _NC_CACHE["nc"]

    in_maps = [{"xt": xt_shards[c], "w": w_dram} for c in range(N_CORES)]
    res = run_bass_kernel_spmd(nc, in_maps, core_ids=list(range(N_CORES)))
    return _unshard([res.results[c]["out"] for c in range(N_CORES)])


# revision 8
# speedup vs baseline: 121.9157x; 1.1243x over previous
"""Trainium2 Bass kernel for nn_LoRALinear (quantized linear + LoRA).

reference:
    w_dq = quant_dequant_int8_per_row(weight)          # [out, in]
    out  = x @ w_dq.T + (alpha/r) * (x @ la) @ lb      # [T, out]

Math identity used here:
    out = x @ (w_dq.T + 2.0 * (la @ lb)) = x @ W_eff

The quant-dequant + LoRA fold is cheap elementwise/skinny-matmul host prep;
the 550-GFLOP dense matmul runs on 8 NeuronCores, data-parallel over tokens.

Device kernel (per core, SPMD identical program):
    xt  [128, 8, 32, 256]  bf16  - token shard, [p][mc][ko][mi] layout
    w   [8, 128, 32, 512]  bf16  - W_eff, [n0][p][ko][ns] layout, replicated
    out [8, 4, 128, 4, 512] bf16 - [n0][g][p][m4][ns] layout

All DRAM<->SBUF transfers are per-partition contiguous (8-32 KiB per
descriptor) so DMA runs at line rate instead of being descriptor-bound.
Matmuls accumulate in fp32 PSUM; the bf16 store adds ~1e-3 relative
rounding on top of the ~2e-3 from bf16 operands (gate is 2e-2).
"""

import numpy as np
import ml_dtypes

TOKENS, IN_F, OUT_F, R = 16384, 4096, 4096, 16
N_CORES = 8
TPC = TOKENS // N_CORES  # tokens per core: 2048
SCALING = 2.0  # alpha / r
P = 128
NS = 512   # out_feature stripe (one PSUM bank of f32)
MC = 256   # x fill chunk: tokens per chunk
MG = 4     # m-tiles per output store group

_NC_CACHE = {}


def _build_nc(tpc=TPC, in_f=IN_F, out_f=OUT_F, ns=NS):
    import concourse.mybir as mybir
    import concourse.tile as tile
    from concourse import bacc

    nc = bacc.Bacc("TRN2", target_bir_lowering=False)

    ko_n = in_f // P    # k-outer tiles (32)
    mt_n = tpc // P     # token tiles (16)
    nt_n = out_f // ns  # out_f stripes (8)
    mc_n = tpc // MC    # x fill chunks (8)
    mg_n = mt_n // MG   # output store groups per stripe (2)

    xt = nc.dram_tensor(
        "xt", [P, mc_n, ko_n, MC], mybir.dt.bfloat16, kind="ExternalInput"
    )
    w = nc.dram_tensor(
        "w", [nt_n, P, ko_n, ns], mybir.dt.bfloat16, kind="ExternalInput"
    )
    out = nc.dram_tensor(
        "out", [nt_n, mg_n, P, MG, ns], mybir.dt.bfloat16, kind="ExternalOutput"
    )

    with tile.TileContext(nc) as tc:
        with (
            tc.tile_pool(name="xpool", bufs=1) as xpool,
            tc.tile_pool(name="wpool", bufs=2) as wpool,
            tc.tile_pool(name="opool", bufs=2) as opool,
            tc.tile_pool(name="pspool", bufs=8, space="PSUM") as pspool,
        ):
            # Whole x shard stays resident in SBUF (bf16: 128 KiB/partition).
            x_sb = xpool.tile([P, mc_n, ko_n, MC], mybir.dt.bfloat16)

            # Stripe 0 of W and the x chunks are interleaved so the first
            # psum group's matmuls wait only on the first chunks, not the
            # whole 21 MB: PE starts a few us in, DMA streams under compute.
            kc_n = 8  # stripe-0 ko chunks
            kcs = ko_n // kc_n
            w_sb0 = wpool.tile([P, ko_n, ns], mybir.dt.bfloat16, name="w_sb")
            issue = []
            for i in range(max(kc_n, mc_n)):
                if i < kc_n:
                    issue.append(("w0", i))
                if i < mc_n:
                    issue.append(("x", i))
            for kind, i in issue:
                if kind == "w0":
                    nc.sync.dma_start(
                        w_sb0[:, i * kcs : (i + 1) * kcs, :],
                        w[0, :, i * kcs : (i + 1) * kcs, :],
                    )
                else:
                    nc.sync.dma_start(x_sb[:, i], xt[:, i])

            for n in range(nt_n):
                if n == 0:
                    w_sb = w_sb0
                else:
                    w_sb = wpool.tile([P, ko_n, ns], mybir.dt.bfloat16, name="w_sb")
                    nc.sync.dma_start(w_sb[:], w[n])
                for g in range(mg_n):
                    o_sb = opool.tile([P, MG, ns], mybir.dt.bfloat16)
                    for m8 in range(MG):
                        m = g * MG + m8
                        mc, mi = divmod(m * P, MC)
                        ps = pspool.tile([P, ns], mybir.dt.float32)
                        for ko in range(ko_n):
                            nc.tensor.matmul(
                                ps[:],
                                x_sb[:, mc, ko, mi : mi + P],
                                w_sb[:, ko, :],
                                start=(ko == 0),
                                stop=(ko == ko_n - 1),
                            )
                        nc.vector.tensor_copy(o_sb[:, m8, :], ps[:])
                    nc.sync.dma_start(out[n, g], o_sb[:])

    nc.finalize()
    return nc


def _host_prep(x, weight, lora_a, lora_b, ns=NS):
    x = np.asarray(x, dtype=np.float32)
    weight = np.asarray(weight, dtype=np.float32)
    la = np.asarray(lora_a, dtype=np.float32)
    lb = np.asarray(lora_b, dtype=np.float32)

    # Symmetric per-row absmax int8 quant-dequant, matching the reference's
    # fp32 elementwise ops bit-for-bit (max/div/round/clip are exact or
    # correctly rounded in IEEE f32 on any backend).
    abs_max = np.max(np.abs(weight), axis=-1, keepdims=True)
    scale = (abs_max / np.float32(127.0)).astype(np.float32)
    wq = np.clip(
        np.round(weight / (scale + np.float32(1e-8))), -128.0, 127.0
    ).astype(np.float32)
    w_dq = wq * scale

    w_eff = w_dq.T + np.float32(SCALING) * (la @ lb)
    w_bf = w_eff.astype(ml_dtypes.bfloat16)
    # [in, out] -> [n0][p][ko][ns]
    w_dram = np.ascontiguousarray(
        w_bf.reshape(IN_F // P, P, OUT_F // ns, ns).transpose(2, 1, 0, 3)
    )

    x_bf = x.astype(ml_dtypes.bfloat16)
    xt_shards = []
    for c in range(N_CORES):
        xs = x_bf[c * TPC : (c + 1) * TPC]  # [tpc, in_f]
        # [tpc, in_f] -> [p][mc][ko][mi]
        xt_shards.append(
            np.ascontiguousarray(
                xs.reshape(TPC // MC, MC, IN_F // P, P).transpose(3, 0, 2, 1)
            )
        )
    return xt_shards, w_dram


def _unshard(outs):
    # outs: per-core [n0][g][p][m8][ns] bf16 -> full [TOKENS, OUT_F] f32
    full = np.empty((TOKENS, OUT_F), dtype=np.float32)
    for c, o in enumerate(outs):
        y = o.transpose(1, 3, 2, 0, 4).reshape(TPC, OUT_F)
        full[c * TPC : (c + 1) * TPC] = y.astype(np.float32)
    return full


def kernel(x, weight, lora_a, lora_b):
    from concourse.bass_utils import run_bass_kernel_spmd

    xt_shards, w_dram = _host_prep(x, weight, lora_a, lora_b)

    if "nc" not in _NC_CACHE:
        _NC_CACHE["nc"] = _build_nc()
    nc = _NC_CACHE["nc"]

    in_maps = [{"xt": xt_shards[c], "w": w_dram} for c in range(N_CORES)]
    res = run_bass_kernel_spmd(nc, in_maps, core_ids=list(range(N_CORES)))
    return _unshard([res.results[c]["out"] for c in range(N_CORES)])
